# revision 1
# baseline (speedup 1.0000x reference)
"""Trainium2 Bass kernel for nn_CausalAttention (B=8, S=2048, D=1024, fp32).

Reference semantics (note: softmax over the QUERY axis, axis=1):
    q = x @ Wq; k = x @ Wk; v = x @ Wv          per batch  [S, D]
    scores[q_, k_] = q[q_] . k[k_], masked to -inf where k_ > q_
    w = softmax(scores, axis=q_)                 (normalize over queries per key)
    out[q_] = sum_k w[q_, k_] v[k_]

Sharding: data-parallel over batch — 8 batches on 8 NeuronCores, QKV weights
replicated, no collectives. Each core runs the identical NEFF on its own batch.

Per-core algorithm (all layouts chosen so softmax runs along the free axis):
  A1: PE-transpose x -> xT[d, s] in SBUF
  A2: Qt[e, q] = Wq^T-as-lhsT x xT      -> DRAM scratch   (fp32r matmuls)
  A3: Kt[e, k]                          -> DRAM scratch
  A4: V[s, e]  = xT-as-lhsT x Wv        -> SBUF resident
  B:  k-outer over k-chunks of 128 (all 4 Qt 512-groups SBUF-resident):
        St[k, q] = Kt-chunk^T-as-lhsT x Qt  (PSUM, fp32r, N=512 groups)
        diag mask add; M = global row-max (negated reduces + min-combine);
        E = exp(St - M) -> bf16 -> DRAM; row-sums via activation accum_out;
        r[k] = 1/sum; V''[k] = r[k] * V[k] (bf16, cached for k-chunks 0-5)
  C:  for each 256-wide q-group:
        out[q-chunk] = sum_k E[k, q]^T-as-lhsT x V''[k]   (bf16 matmuls)

The harness calls kernel(**inputs) with the FULL inputs and expects the FULL
output [8, 2048, 1024] fp32.
"""

import numpy as np

B, S, D = 8, 2048, 1024
P = 128
NCORES = 8
NSC = S // P  # 16 s/k/q chunks of 128
NDC = D // P  # 8 d-chunks
NEC = D // P  # 8 e-chunks
QG = 512      # B-phase q-group width
NQG = S // QG  # 4
CG = 256      # C-phase q-group width (2 q-chunks, nested inside a B group)
NCG = S // CG  # 8
MASK_NEG = -1.0e30
_PHASE_LIMIT = None  # dev: stop build_body after a phase ("A1","A23","A4","B")


def build_body(tc, out_ap, x_ap, wq_ap, wk_ap, wv_ap):
    """Emit the full per-core program into TileContext tc."""
    from contextlib import ExitStack
    import concourse.mybir as mybir
    from concourse.masks import make_identity

    f32 = mybir.dt.float32
    f32r = mybir.dt.float32r
    bf16 = mybir.dt.bfloat16
    AF = mybir.ActivationFunctionType
    ALU = mybir.AluOpType
    AX = mybir.AxisListType

    nc = tc.nc

    with ExitStack() as ctx:
        dram = ctx.enter_context(tc.tile_pool(name="dram", bufs=1, space="DRAM"))
        persist = ctx.enter_context(tc.tile_pool(name="persist", bufs=1))
        xvpool = ctx.enter_context(tc.tile_pool(name="xv", bufs=4))
        wpool = ctx.enter_context(tc.tile_pool(name="w1024", bufs=8))
        qtpool = ctx.enter_context(tc.tile_pool(name="qt", bufs=4))
        ktpool = ctx.enter_context(tc.tile_pool(name="kt", bufs=3))
        stpool = ctx.enter_context(tc.tile_pool(name="stage", bufs=3))
        eopool = ctx.enter_context(tc.tile_pool(name="eout", bufs=3))
        einpool = ctx.enter_context(tc.tile_pool(name="ein", bufs=4))
        vcpool = ctx.enter_context(tc.tile_pool(name="vcache", bufs=5))
        vppool = ctx.enter_context(tc.tile_pool(name="vpp", bufs=2))
        ospool = ctx.enter_context(tc.tile_pool(name="ostage", bufs=2))
        tiny = ctx.enter_context(tc.tile_pool(name="tiny", bufs=4))
        ps512 = ctx.enter_context(tc.tile_pool(name="ps512", bufs=8, space="PSUM"))

        # DRAM scratch, laid out partition-major so B/C-phase DMAs are simple
        qt_dram = dram.tile([P, NEC, S], f32r, tag="qt_d")   # Qt[e%128, e//128, q]
        kt_dram = dram.tile([P, NEC, S], f32r, tag="kt_d")   # Kt[e%128, e//128, k]
        e_dram = dram.tile([P, NSC, S], bf16, tag="e_d")    # E[k%128, k//128, q]

        # constants
        ident_f32 = persist.tile([P, P], f32, tag="ident_f32")
        make_identity(nc, ident_f32[:])
        ident = persist.tile([P, P], f32r, tag="ident")
        nc.vector.tensor_copy(ident[:], ident_f32[:])
        dmask = persist.tile([P, P], f32, tag="dmask")
        # dmask[k, q] = 0 where q >= k else MASK_NEG  (additive causal mask,
        # applied to the diagonal 128x128 tile of St)
        nc.gpsimd.memset(dmask[:], 0.0)
        nc.gpsimd.affine_select(
            out=dmask[:],
            in_=dmask[:],
            compare_op=ALU.is_ge,
            fill=MASK_NEG,
            base=0,
            pattern=[[1, P]],
            channel_multiplier=-1,
        )

        # softmax normalizers: rall[:, kc] = 1 / sum_q exp(s - M) for k-chunk kc
        rall = persist.tile([P, NSC], f32, tag="rall")

        def copy_engine(i):
            return nc.scalar.copy if i % 2 == 0 else nc.vector.tensor_copy

        # ---------------- A1: transpose x -> xT ----------------
        xTg = []
        for g in range(NQG):
            xts = []
            for j in range(4):
                sc = 4 * g + j
                xt = wpool.tile([P, D], f32r, tag="w")
                nc.sync.dma_start(xt[:], x_ap[sc * P:(sc + 1) * P, :])
                xts.append(xt)
            xT = xvpool.tile([P, NDC, QG], f32r, tag="xv")  # xT[d%128, d//128, s in group]
            xTg.append(xT)
            for dc in range(NDC):
                # pack 4 transposed 128x128 blocks into one PSUM bank, one copy out
                pst = ps512.tile([P, QG], f32r, tag="mm", name="pstr")
                for j in range(4):
                    nc.tensor.transpose(pst[:, j * P:(j + 1) * P],
                                        xts[j][:, dc * P:(dc + 1) * P], ident[:])
                copy_engine(dc)(xT[:, dc, :], pst[:])

        if _PHASE_LIMIT == "A1":
            return
        # ---------------- A2/A3: Qt, Kt projections -> DRAM ----------------
        for w_ap, dst in ((wq_ap, qt_dram), (wk_ap, kt_dram)):
            wt = []
            for dc in range(NDC):
                t = wpool.tile([P, D], f32r, tag="w")
                nc.sync.dma_start(t[:], w_ap[dc * P:(dc + 1) * P, :])
                wt.append(t)
            for ec in range(NEC):
                pss = [ps512.tile([P, QG], f32, tag="mm", name=f"psproj{g}")
                       for g in range(NQG)]
                for dc in range(NDC):
                    lhs = wt[dc][:, ec * P:(ec + 1) * P]
                    for g in range(NQG):
                        nc.tensor.matmul(
                            pss[g][:], lhs, xTg[g][:, dc, :],
                            start=(dc == 0), stop=(dc == NDC - 1),
                        )
                for g in range(NQG):
                    st = stpool.tile([P, QG], f32r, tag="st")
                    copy_engine(ec + g)(st[:], pss[g][:])
                    nc.sync.dma_start(dst[:, ec, g * QG:(g + 1) * QG], st[:])

        if _PHASE_LIMIT == "A23":
            return
        # ---------------- A4: V projection -> SBUF (resident) ----------------
        wt = []
        for dc in range(NDC):
            t = wpool.tile([P, D], f32r, tag="w")
            nc.sync.dma_start(t[:], wv_ap[dc * P:(dc + 1) * P, :])
            wt.append(t)
        v_tiles = []
        for g in range(NQG):
            vt = xvpool.tile([P, 4, D], f32, tag="xv")  # V[s%128, s-chunk in group, e]
            v_tiles.append(vt)
            for jp in range(0, 4, 2):  # s-chunk pairs -> 4 PSUM banks in flight
                pp = [ps512.tile([P, QG], f32, tag="mm", name=f"psv{j}_{eh}")
                      for j in range(2) for eh in range(2)]
                for dc in range(NDC):
                    for j in range(2):
                        lhs = xTg[g][:, dc, (jp + j) * P:(jp + j + 1) * P]
                        nc.tensor.matmul(pp[2 * j][:], lhs, wt[dc][:, 0:QG],
                                         start=(dc == 0), stop=(dc == NDC - 1))
                        nc.tensor.matmul(pp[2 * j + 1][:], lhs, wt[dc][:, QG:D],
                                         start=(dc == 0), stop=(dc == NDC - 1))
                for j in range(2):
                    copy_engine(j)(vt[:, jp + j, 0:QG], pp[2 * j][:])
                    copy_engine(j + 1)(vt[:, jp + j, QG:D], pp[2 * j + 1][:])

        if _PHASE_LIMIT == "A4":
            return
        # ---------------- B: scores + exp + stats (k-outer) ----------------
        # All 4 Qt q-groups resident (loaded once, hidden under A3/A4); per
        # k-chunk the global row-max over all valid q is available in one pass,
        # so E = exp(s - M) needs no later correction and r folds into V once.
        qts = []
        for qg in range(NQG):
            qt_t = qtpool.tile([P, NEC, QG], f32r, tag="qt", name=f"qt{qg}")
            nc.sync.dma_start(qt_t[:], qt_dram[:, :, qg * QG:(qg + 1) * QG])
            qts.append(qt_t)
        vcache = {}
        for kc in range(NSC):
            g0 = kc // 4
            kt_t = ktpool.tile([P, NEC, P], f32r, tag="kt")
            nc.sync.dma_start(kt_t[:], kt_dram[:, :, kc * P:(kc + 1) * P])
            pss = {qg: ps512.tile([P, QG], f32, tag="mm", name=f"pssc{qg}")
                   for qg in range(g0, NQG)}
            for dc in range(NEC):
                lhs = kt_t[:, dc, :]
                for qg in range(g0, NQG):
                    nc.tensor.matmul(
                        pss[qg][:], lhs, qts[qg][:, dc, :],
                        start=(dc == 0), stop=(dc == NEC - 1),
                    )
            off0 = (kc % 4) * P
            nc.vector.tensor_tensor(
                pss[g0][:, off0:off0 + P], pss[g0][:, off0:off0 + P], dmask[:],
                ALU.add,
            )
            nmall = tiny.tile([P, NQG], f32, tag="nmall")
            for qg in range(g0, NQG):
                off = off0 if qg == g0 else 0
                nc.vector.tensor_reduce(nmall[:, qg:qg + 1], pss[qg][:, off:QG],
                                        axis=AX.X, op=ALU.max, negate=True)
            negM = tiny.tile([P, 1], f32, tag="negM")
            nc.vector.tensor_reduce(negM[:], nmall[:, g0:NQG], axis=AX.X,
                                    op=ALU.min)
            sums = tiny.tile([P, NQG], f32, tag="sums")
            for qg in range(g0, NQG):
                off = off0 if qg == g0 else 0
                et = eopool.tile([P, QG], bf16, tag="et")
                nc.scalar.activation(et[:, off:QG], pss[qg][:, off:QG], AF.Exp,
                                     bias=negM[:], scale=1.0,
                                     accum_out=sums[:, qg:qg + 1])
                nc.sync.dma_start(
                    e_dram[:, kc, qg * QG + off:(qg + 1) * QG], et[:, off:QG]
                )
            ssum = tiny.tile([P, 1], f32, tag="ssum")
            nc.vector.tensor_reduce(ssum[:], sums[:, g0:NQG], axis=AX.X, op=ALU.add)
            nc.vector.reciprocal(rall[:, kc:kc + 1], ssum[:])
            if kc < 5:
                # pre-scale V rows by r for the high-reuse k-chunks
                vc = vcpool.tile([P, D], bf16, tag="vc", name=f"vc{kc}")
                nc.vector.tensor_scalar_mul(
                    vc[:], v_tiles[kc // 4][:, kc % 4, :], rall[:, kc:kc + 1]
                )
                vcache[kc] = vc

        if _PHASE_LIMIT == "B":
            return
        # ---------------- C: out = E^T x (r * V) ----------------
        for cgi in range(NCG):
            qcs = (2 * cgi, 2 * cgi + 1)
            pso = {qc: [ps512.tile([P, QG], f32, tag="mm", name=f"psav{qc}_{eh}")
                        for eh in range(2)]
                   for qc in qcs}
            for kc in range(2 * cgi + 2):
                ec_t = einpool.tile([P, CG], bf16, tag="ein")
                nc.sync.dma_start(ec_t[:], e_dram[:, kc, cgi * CG:(cgi + 1) * CG])
                if kc in vcache:
                    vpp = vcache[kc]
                else:
                    vpp = vppool.tile([P, D], bf16, tag="vpp")
                    nc.vector.tensor_scalar_mul(
                        vpp[:], v_tiles[kc // 4][:, kc % 4, :], rall[:, kc:kc + 1]
                    )
                for qi, qc in enumerate(qcs):
                    if qc < kc:
                        continue
                    for eh in range(2):
                        nc.tensor.matmul(
                            pso[qc][eh][:],
                            ec_t[:, qi * P:(qi + 1) * P],
                            vpp[:, eh * QG:(eh + 1) * QG],
                            start=(kc == 0), stop=(kc == qc),
                        )
            for qi, qc in enumerate(qcs):
                st = ospool.tile([P, D], f32, tag="os")
                copy_engine(qi)(st[:, 0:QG], pso[qc][0][:])
                copy_engine(qi + 1)(st[:, QG:D], pso[qc][1][:])
                nc.sync.dma_start(out_ap[qc * P:(qc + 1) * P, :], st[:])


_PROGRAMS = {}


def _get_program(n_repeats=1):
    if n_repeats not in _PROGRAMS:
        from concourse import bacc
        import concourse.tile as tile
        import concourse.mybir as mybir

        f32 = mybir.dt.float32
        nc = bacc.Bacc("TRN2", target_bir_lowering=False, debug=False,
                       enable_asserts=False, num_devices=NCORES)
        x_ap = nc.dram_tensor("x_local", (S, D), mybir.dt.float32r, kind="ExternalInput").ap()
        wq_ap = nc.dram_tensor("wq", (D, D), mybir.dt.float32r, kind="ExternalInput").ap()
        wk_ap = nc.dram_tensor("wk", (D, D), mybir.dt.float32r, kind="ExternalInput").ap()
        wv_ap = nc.dram_tensor("wv", (D, D), mybir.dt.float32r, kind="ExternalInput").ap()
        out_ap = nc.dram_tensor("out_local", (S, D), f32, kind="ExternalOutput").ap()
        with tile.TileContext(nc) as tc:
            if n_repeats == 1:
                build_body(tc, out_ap, x_ap, wq_ap, wk_ap, wv_ap)
            else:
                with tc.For_i(0, n_repeats, 1):
                    build_body(tc, out_ap, x_ap, wq_ap, wk_ap, wv_ap)
        nc.compile()
        _PROGRAMS[n_repeats] = nc
    return _PROGRAMS[n_repeats]


def run(x, Wq, Wk, Wv, trace=False, **spmd_kwargs):
    """Run on all 8 cores; returns (out [8,S,D] fp32, BassKernelResults)."""
    from concourse import bass_utils

    nc = _get_program()
    x = np.ascontiguousarray(np.asarray(x, dtype=np.float32))
    Wq = np.ascontiguousarray(np.asarray(Wq, dtype=np.float32))
    Wk = np.ascontiguousarray(np.asarray(Wk, dtype=np.float32))
    Wv = np.ascontiguousarray(np.asarray(Wv, dtype=np.float32))
    in_maps = [
        {"x_local": np.ascontiguousarray(x[i]), "wq": Wq, "wk": Wk, "wv": Wv}
        for i in range(NCORES)
    ]
    res = bass_utils.run_bass_kernel_spmd(
        nc, in_maps, core_ids=list(range(NCORES)), trace=trace, **spmd_kwargs
    )
    out = np.stack([r["out_local"] for r in res.results]).astype(np.float32)
    return out, res


def kernel(x, Wq, Wk, Wv):
    out, _ = run(x, Wq, Wk, Wv, trace=False)
    return out



# revision 3
# speedup vs baseline: 26.3619x; 26.3619x over previous
"""Trainium2 Bass kernel v2 for nn_CausalAttention (B=8, S=2048, D=1024, fp32).

Reference semantics (softmax over the QUERY axis, axis=1):
    q = x @ Wq; k = x @ Wk; v = x @ Wv          per batch  [S, D]
    scores[q_, k_] = q[q_] . k[k_], masked to -inf where k_ > q_
    w = softmax(scores, axis=q_)                 (normalize over queries per key)
    out[q_] = sum_k w[q_, k_] v[k_]

v2 (data-parallel over batch, 8 cores, no collectives). Per core:
    S^T = K Q^T = (x M') x^T with M' = Wk Wq^T   (one fewer projection GEMM)
  Prologue: Wq/Wk -> PE-transpose -> M' (fp32r, SBUF); x -> PE-transpose xT.
  Fused main loop over k-chunks kc (128 k rows):
    - every 4th kc: A'T[j, k-512-group] = M'^T @ xT        (SBUF, 2MB)
    - St[k, q] = A'T_kc^T @ xT  (q-512 groups, causal-skipped, diagonal
      group N-trimmed), diag mask, global row max, E = exp(St-M) -> bf16
      -> one whole-row DMA to DRAM; row-sums via activation accum_out;
      r = 1/sum
    - V[kc] = xT_kc^T @ Wv (psum), V''[kc] = r*V -> bf16 (SBUF resident)
    - lagged by 2: C-group j: out[q-chunk] = sum_kc E^T @ V'' (bf16)
  SBUF: one rotating 32KB-slab tag {Wq,Wk,WqT,WkT -> M',Wv,V''} keeps
  peak under the ~208KB/partition budget.
"""

import numpy as np

B, S, D = 8, 2048, 1024
P = 128
NCORES = 8
NSC = S // P   # 16 k/q chunks of 128
NDC = D // P   # 8 d-chunks
QG = 512       # B-phase q-group width
NQG = S // QG  # 4
CG = 256       # C-phase q-block width per group (2 q-chunks)
MASK_NEG = -1.0e30


def build_body(tc, out_ap, x_ap, wq_ap, wk_ap, wv_ap):
    from contextlib import ExitStack
    import concourse.mybir as mybir
    from concourse.masks import make_identity

    f32 = mybir.dt.float32
    f32r = mybir.dt.float32r
    bf16 = mybir.dt.bfloat16
    AF = mybir.ActivationFunctionType
    ALU = mybir.AluOpType
    AX = mybir.AxisListType

    nc = tc.nc

    with ExitStack() as ctx:
        dram = ctx.enter_context(tc.tile_pool(name="dram", bufs=1, space="DRAM"))
        persist = ctx.enter_context(tc.tile_pool(name="persist", bufs=1))
        w32 = ctx.enter_context(tc.tile_pool(name="w32", bufs=3))
        xck = ctx.enter_context(tc.tile_pool(name="xck", bufs=3))
        xtp = ctx.enter_context(tc.tile_pool(name="xtp", bufs=1))
        atp = ctx.enter_context(tc.tile_pool(name="atp", bufs=1))
        etp = ctx.enter_context(tc.tile_pool(name="etp", bufs=1))
        ecp = ctx.enter_context(tc.tile_pool(name="ecp", bufs=1))
        osp = ctx.enter_context(tc.tile_pool(name="osp", bufs=1))
        tiny = ctx.enter_context(tc.tile_pool(name="tiny", bufs=4))
        ps512 = ctx.enter_context(tc.tile_pool(name="ps512", bufs=8, space="PSUM"))

        e_dram = dram.tile([P, NSC, S], bf16, tag="e_d")  # E[k%128, k//128, q]

        # constants
        ident_f32 = persist.tile([P, P], f32, tag="ident_f32")
        make_identity(nc, ident_f32[:])
        ident = persist.tile([P, P], f32r, tag="ident")
        nc.vector.tensor_copy(ident[:], ident_f32[:])
        dmask = persist.tile([P, P], f32, tag="dmask")
        # dmask[k, q] = 0 where q >= k else MASK_NEG
        nc.gpsimd.memset(dmask[:], 0.0)
        nc.gpsimd.affine_select(
            out=dmask[:], in_=dmask[:], compare_op=ALU.is_ge, fill=MASK_NEG,
            base=0, pattern=[[1, P]], channel_multiplier=-1,
        )
        rall = persist.tile([P, NSC], f32, tag="rall")

        def copy_engine(i):
            return nc.scalar.copy if i % 2 == 0 else nc.vector.tensor_copy

        # PE warmup: release the HAM clock gate (~3.4us of activity) while
        # the W DMAs are still in flight; junk transposes, never read.
        for w in range(6):
            pwu = ps512.tile([P, QG], f32r, tag="mm", name="pwu")
            for j in range(4):
                nc.tensor.transpose(pwu[:, j * P:(j + 1) * P], ident[:],
                                    ident[:])

        # ---------------- loads: Wq, Wk first (PE needs them earliest) -----
        wq_t = w32.tile([P, NDC, D], f32r, tag="w32", name="wq_t")   # slot0
        for dc in range(NDC):
            nc.sync.dma_start(wq_t[:, dc, :], wq_ap[dc * P:(dc + 1) * P, :])
        wk_t = w32.tile([P, NDC, D], f32r, tag="w32", name="wk_t")   # slot1
        for dc in range(NDC):
            nc.sync.dma_start(wk_t[:, dc, :], wk_ap[dc * P:(dc + 1) * P, :])

        # ---------------- W transposes: wXT[e%128, e//128, i] --------------
        def transpose_w(wsrc, wdst):
            # half-outer: all ec of dc 0-3 first, so PE never head-of-line
            # blocks on the second half of the W DMA
            for half in range(2):
                for ec in range(NDC):
                    pst = ps512.tile([P, QG], f32r, tag="mm", name="pstw")
                    for j in range(4):
                        dc = half * 4 + j
                        nc.tensor.transpose(
                            pst[:, j * P:(j + 1) * P],
                            wsrc[:, dc, ec * P:(ec + 1) * P], ident[:])
                    copy_engine(ec + half)(
                        wdst[:, ec, half * QG:(half + 1) * QG], pst[:])

        wqT = w32.tile([P, NDC, D], f32r, tag="w32", name="wqT")     # slot2
        transpose_w(wq_t, wqT)
        wkT = w32.tile([P, NDC, D], f32r, tag="w32", name="wkT")     # slot0
        transpose_w(wk_t, wkT)

        # ------- M' chains interleaved with x load + transpose -------------
        # M'[i, j] = sum_e Wk[i, e] Wq[j, e]; xT[d%128, d//128, s].
        # Interleaving keeps the x-chunk DMA pipeline draining (bufs=3
        # rotation frees a chunk right after its transposes) while PE chews
        # on M' accumulation chains.
        mp = w32.tile([P, NDC, D], f32r, tag="w32", name="mp")       # slot1
        xT = xtp.tile([P, NDC, S], f32r, tag="xt")

        def emit_x_chunk(sc):
            c = xck.tile([P, D], f32r, tag="xc", name="xc")
            nc.sync.dma_start(c[:], x_ap[sc * P:(sc + 1) * P, :])
            for half in range(2):
                pst = ps512.tile([P, QG], f32r, tag="mm", name="pstx")
                for j in range(4):
                    dc = half * 4 + j
                    nc.tensor.transpose(pst[:, j * P:(j + 1) * P],
                                        c[:, dc * P:(dc + 1) * P], ident[:])
                for j in range(4):
                    dc = half * 4 + j
                    copy_engine(sc + j)(xT[:, dc, sc * P:(sc + 1) * P],
                                        pst[:, j * P:(j + 1) * P])

        for t in range(16):
            ic, jg = t // 2, t % 2
            psm = ps512.tile([P, QG], f32, tag="mm", name="psm")
            for ec in range(NDC):
                nc.tensor.matmul(
                    psm[:], wkT[:, ec, ic * P:(ic + 1) * P],
                    wqT[:, ec, jg * QG:(jg + 1) * QG],
                    start=(ec == 0), stop=(ec == NDC - 1),
                )
            copy_engine(ic + jg)(mp[:, ic, jg * QG:(jg + 1) * QG], psm[:])
            emit_x_chunk(t)

        # Wv load (SP queue: after x chunks; lands before V(0) is needed)
        wv_t = w32.tile([P, NDC, D], f32r, tag="w32", name="wv_t")   # slot2
        for dc in range(NDC):
            nc.sync.dma_start(wv_t[:, dc, :], wv_ap[dc * P:(dc + 1) * P, :])
        vpp_t = w32.tile([P, NSC, D], bf16, tag="w32", name="vpp_t")  # slot0

        # ---------------- main fused loop over k-chunks ----------------
        at_t = None
        ec_t = None

        def emit_at_group(g):
            t = atp.tile([P, NDC, QG], f32r, tag="at", name=f"at{g}")
            for jc in range(NDC):
                ps = ps512.tile([P, QG], f32, tag="mm", name="psat")
                for ic in range(NDC):
                    nc.tensor.matmul(
                        ps[:], mp[:, ic, jc * P:(jc + 1) * P],
                        xT[:, ic, g * QG:(g + 1) * QG],
                        start=(ic == 0), stop=(ic == NDC - 1),
                    )
                copy_engine(jc)(t[:, jc, :], ps[:])
            return t

        def emit_c_group(j, ec_t, qis=(0, 1)):
            # out[q-chunk qc] = sum_{kc<=qc} E[kc block]^T @ V''[kc]
            for qi in qis:
                qc = 2 * j + qi
                pso = [ps512.tile([P, QG], f32, tag="mm", name=f"psc{eh}")
                       for eh in range(2)]
                for kc in range(qc + 1):
                    for eh in range(2):
                        nc.tensor.matmul(
                            pso[eh][:], ec_t[:, kc, qi * P:(qi + 1) * P],
                            vpp_t[:, kc, eh * QG:(eh + 1) * QG],
                            start=(kc == 0), stop=(kc == qc),
                        )
                st = osp.tile([P, D], f32, tag="os", name="ost")
                copy_engine(qi)(st[:, 0:QG], pso[0][:])
                copy_engine(qi + 1)(st[:, QG:D], pso[1][:])
                nc.sync.dma_start(out_ap[qc * P:(qc + 1) * P, :], st[:])

        # C-group schedule: one group per kc, staggered to avoid the A'T
        # PSUM bursts at kc % 4 == 0; C(7) split so only qc=15 trails.
        c_sched = {2: 0, 5: 1, 6: 2, 9: 3, 10: 4, 13: 5, 14: 6}
        at_t = emit_at_group(0)
        for kc in range(NSC):
            g0 = kc // 4
            off0 = (kc % 4) * P
            if kc % 4 == 0 and kc > 0:
                at_t = emit_at_group(g0)
            # scores St[k, q] for q >= kc*128, q-512 groups; diagonal group
            # trimmed to >=256 columns (fp32r full-rate threshold)
            off_mm0 = min(off0, QG - 2 * P)
            pss = {}
            for qg in range(g0, NQG):
                off = off_mm0 if qg == g0 else 0
                ps = ps512.tile([P, QG], f32, tag="mm", name=f"pssc{qg}")
                pss[qg] = ps
                for jc in range(NDC):
                    nc.tensor.matmul(
                        ps[:, off:QG],
                        at_t[:, jc, off0:off0 + P],
                        xT[:, jc, qg * QG + off:(qg + 1) * QG],
                        start=(jc == 0), stop=(jc == NDC - 1),
                    )
            # C-group compute: only needs E rows <= kc-1 and V'' <= kc-1
            if kc in c_sched:
                emit_c_group(c_sched[kc], ec_t)
            if kc == NSC - 1:
                # prefetch E rows 0-14 for q-chunks 14/15 (row 15 after exp)
                ec_t = ecp.tile([P, NSC, CG], bf16, tag="ec", name="ec_t")
                nc.sync.dma_start(ec_t[:, 0:NSC - 1, :],
                                  e_dram[:, 0:NSC - 1, 7 * CG:8 * CG])
            # diagonal mask + global row max (negated max, min-combined)
            nc.vector.tensor_tensor(
                pss[g0][:, off0:off0 + P], pss[g0][:, off0:off0 + P], dmask[:],
                ALU.add,
            )
            nmall = tiny.tile([P, NQG], f32, tag="nmall")
            for qg in range(g0, NQG):
                off = off0 if qg == g0 else 0
                nc.vector.tensor_reduce(nmall[:, qg:qg + 1], pss[qg][:, off:QG],
                                        axis=AX.X, op=ALU.max, negate=True)
            negM = tiny.tile([P, 1], f32, tag="negM")
            nc.vector.tensor_reduce(negM[:], nmall[:, g0:NQG], axis=AX.X,
                                    op=ALU.min)
            # E = exp(s - M) -> bf16, row sums accumulated; one row DMA
            et = etp.tile([P, S], bf16, tag="et", name="et")
            sums = tiny.tile([P, NQG], f32, tag="sums")
            for qg in range(g0, NQG):
                off = off0 if qg == g0 else 0
                nc.scalar.activation(et[:, qg * QG + off:(qg + 1) * QG],
                                     pss[qg][:, off:QG], AF.Exp,
                                     bias=negM[:], scale=1.0,
                                     accum_out=sums[:, qg:qg + 1])
            nc.sync.dma_start(e_dram[:, kc, kc * P:S], et[:, kc * P:S])
            # C-group E prefetch right behind this kc's E-row write
            if kc % 2 == 1 and kc < NSC - 1:
                j = (kc - 1) // 2
                ec_t = ecp.tile([P, NSC, CG], bf16, tag="ec", name="ec_t")
                nc.sync.dma_start(ec_t[:, 0:2 * j + 2, :],
                                  e_dram[:, 0:2 * j + 2, j * CG:(j + 1) * CG])
            ssum = tiny.tile([P, 1], f32, tag="ssum")
            nc.vector.tensor_reduce(ssum[:], sums[:, g0:NQG], axis=AX.X,
                                    op=ALU.add)
            nc.vector.reciprocal(rall[:, kc:kc + 1], ssum[:])
            # V[kc] = xT_kc^T @ Wv; V''[kc] = r * V -> bf16
            for eh in range(2):
                psv = ps512.tile([P, QG], f32, tag="mm", name="psv")
                for dc in range(NDC):
                    nc.tensor.matmul(
                        psv[:], xT[:, dc, kc * P:(kc + 1) * P],
                        wv_t[:, dc, eh * QG:(eh + 1) * QG],
                        start=(dc == 0), stop=(dc == NDC - 1),
                    )
                nc.vector.tensor_scalar_mul(
                    vpp_t[:, kc, eh * QG:(eh + 1) * QG], psv[:],
                    rall[:, kc:kc + 1],
                )
            if kc == NSC - 1:
                # qc=14 needs only E rows 0-14 (prefetched above) and r<=14
                emit_c_group(7, ec_t, qis=(0,))
                nc.sync.dma_start(ec_t[:, NSC - 1:NSC, :],
                                  e_dram[:, NSC - 1:NSC, 7 * CG:8 * CG])

        emit_c_group(7, ec_t, qis=(1,))


_PROGRAMS = {}


def _get_program(n_repeats=1):
    if n_repeats not in _PROGRAMS:
        from concourse import bacc
        import concourse.tile as tile
        import concourse.mybir as mybir

        f32 = mybir.dt.float32
        nc = bacc.Bacc("TRN2", target_bir_lowering=False, debug=False,
                       enable_asserts=False, num_devices=NCORES)
        x_ap = nc.dram_tensor("x_local", (S, D), mybir.dt.float32r, kind="ExternalInput").ap()
        wq_ap = nc.dram_tensor("wq", (D, D), mybir.dt.float32r, kind="ExternalInput").ap()
        wk_ap = nc.dram_tensor("wk", (D, D), mybir.dt.float32r, kind="ExternalInput").ap()
        wv_ap = nc.dram_tensor("wv", (D, D), mybir.dt.float32r, kind="ExternalInput").ap()
        out_ap = nc.dram_tensor("out_local", (S, D), f32, kind="ExternalOutput").ap()
        with tile.TileContext(nc) as tc:
            if n_repeats == 1:
                build_body(tc, out_ap, x_ap, wq_ap, wk_ap, wv_ap)
            else:
                with tc.For_i(0, n_repeats, 1):
                    build_body(tc, out_ap, x_ap, wq_ap, wk_ap, wv_ap)
        nc.compile()
        _PROGRAMS[n_repeats] = nc
    return _PROGRAMS[n_repeats]


def run(x, Wq, Wk, Wv, trace=False, **spmd_kwargs):
    from concourse import bass_utils

    nc = _get_program()
    x = np.ascontiguousarray(np.asarray(x, dtype=np.float32))
    Wq = np.ascontiguousarray(np.asarray(Wq, dtype=np.float32))
    Wk = np.ascontiguousarray(np.asarray(Wk, dtype=np.float32))
    Wv = np.ascontiguousarray(np.asarray(Wv, dtype=np.float32))
    in_maps = [
        {"x_local": np.ascontiguousarray(x[i]), "wq": Wq, "wk": Wk, "wv": Wv}
        for i in range(NCORES)
    ]
    res = bass_utils.run_bass_kernel_spmd(
        nc, in_maps, core_ids=list(range(NCORES)), trace=trace, **spmd_kwargs
    )
    out = np.stack([r["out_local"] for r in res.results]).astype(np.float32)
    return out, res


def kernel(x, Wq, Wk, Wv):
    out, _ = run(x, Wq, Wk, Wv, trace=False)
    return out
